# revision 54
# baseline (speedup 1.0000x reference)
"""Multi-head attention (B=4, S=2048, D=256, H=4) on 8 trn2 NeuronCores.

Sharding: core c handles batch b = c//2 and query half qh = c%2 (1024
queries), all 4 heads, full 2048 keys.  The host rolls x[b].T by
-qh*1024 columns so every core's queries sit at columns 0-1023 of its
xT input (key order is free: softmax+sum over keys is permutation
invariant as long as the mask bias is rolled identically).  This keeps
the SPMD program core-agnostic and avoids a separate xq input.

Per-core dataflow (scores kept transposed: [keys, queries]):
  QT = WQT.T-chunks @ xT[:, :1024]  -> Q.T [256(feat), 1024(q)]
  KT = WKT.T-chunks @ xT            -> K.T [256(feat), 2048(k)]
  V  = xT-chunks.T  @ WVT           -> V_aug [2048(k), 4, 65] (ones col)
  64 (section, key-tile) steps; section = (head pair p, query half f):
     S.T[kt, q] = KT_h-slices.T @ QT-slices  (2 heads row-packed in PE)
     E.T        = exp(S.T * scale + mask_bias[key])      (ScalarE)
     cd_h      += V_aug_h.T @ E.T   (rows 0-63 = ctx.T, row 64 = den;
                                     one PSUM bank per head, accumulated
                                     over the 16 key tiles of a section)
  after each section: cd evicted to SBUF; den row transposed to
  partitions via 8 single-row matmuls -> [128,16] -> DVE reciprocal.
  out tile m: per-head matmuls ps[:,h,:] = ctx_h.T-chunk @ WOT_h, then
  out = sum_h ps[:,h,:] * (1/den_h) via per-partition-scalar
  multiply-adds (normalization folded into the output combine, which is
  valid because each head's 1/den[h,q] scales whole output rows).

The 64 steps are emitted as ONE globally software-pipelined stream:
scores(step i) and cd(step i-4) per iteration.  The PE engine is
in-order, so a cd matmul blocked on its exp ACT -- or, at a section's
first tile, on the previous section's cd eviction COPY -- always has
several independent scores steps queued ahead of it.  Without this the
PE duty cycle dips at section boundaries (it becomes slaved to the
serial ACT drain of the old section) and the HAM activity monitor
re-gates the PE clock to 4/8, costing 10-20us a run.

Matmul operands are float32r (TF32-like, 1 PE cycle/col for N>=256).
fp32r matmuls must write PSUM at partition offset 0 and need even
free counts + 8B-aligned dsts, hence the duplicated-column denominator
transpose matmuls.  Input DMAs are split into pieces spread over the
three DMA-capable engine queues (sync/scalar/gpsimd, ~130GB/s each) in
consumption order, so the first scores matmul starts ~11us in and the
bulk of xT streams in behind the running attention pipeline.
"""

import sys

for _p in ("/opt/trn_rl_repo",):
    if _p not in sys.path:
        sys.path.insert(0, _p)

import numpy as np

B, S, D, H, HD = 4, 2048, 256, 4, 64
SCALE = HD**-0.5
NCORES = 8
QS = S // 2  # queries per core
QH = QS // 2  # query half (one psum bank wide per head)
P = 128
NKT = S // P  # 16 key tiles
SECS = ((0, 0), (1, 0), (0, 1), (1, 1))  # (head pair, query half)
DEPTH = 3  # cd lags scores by this many steps in the PE stream

_cache = {}


def _build_nc():
    import concourse.mybir as mybir
    from concourse import bacc
    from concourse.tile import TileContext

    f32 = mybir.dt.float32
    f32r = mybir.dt.float32r
    Exp = mybir.ActivationFunctionType.Exp
    Copy = mybir.ActivationFunctionType.Copy
    Alu = mybir.AluOpType

    nc = bacc.Bacc("TRN2", target_bir_lowering=False, debug=False)

    xT_d = nc.dram_tensor("xT", [D, S], f32, kind="ExternalInput")
    wqt_d = nc.dram_tensor("wqt", [D, D], f32, kind="ExternalInput")
    wkt_d = nc.dram_tensor("wkt", [D, D], f32, kind="ExternalInput")
    wvt_d = nc.dram_tensor("wvt", [D, D], f32, kind="ExternalInput")
    wot_d = nc.dram_tensor("wot", [D, D], f32, kind="ExternalInput")
    bias_d = nc.dram_tensor("bias", [P, NKT], f32, kind="ExternalInput")
    out_d = nc.dram_tensor("out", [QS, D], f32, kind="ExternalOutput")

    with TileContext(nc) as tc:
        with (
            tc.tile_pool(name="const", bufs=1) as const,
            tc.tile_pool(name="big", bufs=1) as big,
            tc.tile_pool(name="et", bufs=8) as etp,
            tc.tile_pool(name="small", bufs=2) as small,
            tc.tile_pool(name="psA", bufs=3, space="PSUM") as psA,
            tc.tile_pool(name="psCD", bufs=1, space="PSUM") as psCD,
        ):
            # ---- input DMAs: pieces at the head of three hardware queues
            # (each sustains only ~130GB/s) in consumption order.  Critical
            # path to the first scores matmul: wqt/wkt feature block 0 +
            # xT cols 0-511 of both 128-row chunks. ----
            w_sb = {}
            for nm in ("wqt", "wkt", "wvt"):
                w_sb[nm] = const.tile([P, 2, D], f32r, name=f"w_{nm}", tag=f"w_{nm}")

            def w_piece(nm, dram, e0, e1, eng):
                eng.dma_start(
                    out=w_sb[nm][:, :, e0:e1],
                    in_=dram.rearrange("(c p) e -> p c e", p=P)[:, :, e0:e1].bitcast(
                        f32r
                    ),
                )

            xT_sb = []
            for c in range(2):
                xt = big.tile([P, S], f32r, name=f"xT{c}", tag=f"xT{c}")
                xT_sb.append(xt)

            def x_piece(c, k0, k1, eng):
                eng.dma_start(
                    out=xT_sb[c][:, k0:k1],
                    in_=xT_d[c * P : (c + 1) * P, k0:k1].bitcast(f32r),
                )

            bias_sb = const.tile([P, NKT], f32)
            # scalar queue: three small critical pieces, all done ~11us,
            # well before the first EXP needs the engine.
            w_piece("wqt", wqt_d, 0, P, nc.scalar)
            w_piece("wkt", wkt_d, 0, P, nc.scalar)
            nc.scalar.dma_start(out=bias_sb, in_=bias_d[:, :])
            # sync queue: xT chunk 0, first 256 cols leading.
            x_piece(0, 0, 256, nc.sync)
            x_piece(0, 256, 512, nc.sync)
            x_piece(0, 512, 1024, nc.sync)
            x_piece(0, 1024, 1536, nc.sync)
            x_piece(0, 1536, 2048, nc.sync)
            w_piece("wqt", wqt_d, P, D, nc.sync)
            w_piece("wkt", wkt_d, P, D, nc.sync)
            # gpsimd queue: xT chunk 1 + V/O weights.
            x_piece(1, 0, 256, nc.gpsimd)
            x_piece(1, 256, 512, nc.gpsimd)
            w_piece("wvt", wvt_d, 0, D, nc.gpsimd)
            x_piece(1, 512, 1024, nc.gpsimd)
            x_piece(1, 1024, 1536, nc.gpsimd)
            x_piece(1, 1536, 2048, nc.gpsimd)
            # W_O.T grouped per head: [64, 4, 256] so each head's
            # contraction chunk starts at partition 0.
            wot_sb = const.tile([64, 4, D], f32r, name="w_wot", tag="w_wot")
            nc.sync.dma_start(
                out=wot_sb, in_=wot_d.rearrange("(h p) e -> p h e", p=64).bitcast(f32r)
            )

            # ---- constants ----
            ones4 = const.tile([P, 4], f32)
            nc.vector.memset(ones4, 1.0)
            one1 = const.tile([P, 2], f32r)
            nc.vector.tensor_copy(one1, ones4[:, 0:2])

            # PE warm-up: the HAM clock gate keeps the PE at 4/8 until it
            # sees ~4us of sustained activity, and the DMA-paced prologue
            # isn't dense enough, which used to delay the release to ~33us
            # (half-clock matmuls through the first third of s00).  Burn
            # ~4us of wide matmuls on const data (no DMA deps, ~90% duty)
            # before the first projection so the gate opens right as the
            # real pipeline fills.  They write the psCD slot, unused until
            # the first cd accumulation many us later.
            dummy_f = const.tile([P, 512], f32)
            nc.vector.memset(dummy_f, 1.0)
            dummy = const.tile([P, 512], f32r)
            nc.vector.tensor_copy(dummy, dummy_f)
            ps_warm = psCD.tile([2, 512], f32, name="ps_warm", tag="psCD")

            def warm(n):
                for _ in range(n):
                    nc.tensor.matmul(
                        ps_warm[0:2, :], one1, dummy, start=True, stop=True
                    )

            def tail_warm(n):
                # tail-side warm matmuls: ps_warm's psCD slot belongs to a
                # cd accumulator by then, so use a (then idle) psA slot
                wt = psA.tile([2, 512], f32, name="ps_tw", tag="psA")
                for _ in range(n):
                    nc.tensor.matmul(wt[0:2, :], one1, dummy, start=True, stop=True)

            warm(10)

            QT_sb = [None, None]
            KT_sb = [None, None]
            V_sb = [None] * NKT
            cd_sb = {}
            r_sb = {}
            for p in range(2):
                for f in range(2):
                    r_sb[(p, f)] = big.tile(
                        [P, 16], f32, name=f"r{p}{f}", tag=f"r{p}{f}"
                    )

            def qt_proj(m, k0, k1):
                if QT_sb[m] is None:
                    QT_sb[m] = big.tile([P, QS], f32r, name=f"QT{m}", tag=f"QT{m}")
                ps = psA.tile([P, k1 - k0], f32, name="ps_q", tag="psA")
                for c in range(2):
                    nc.tensor.matmul(
                        ps[:, :],
                        w_sb["wqt"][:, c, m * P : (m + 1) * P],
                        xT_sb[c][:, k0:k1],
                        start=(c == 0),
                        stop=(c == 1),
                    )
                nc.vector.tensor_copy(QT_sb[m][:, k0:k1], ps)

            def kt_proj(m, k0, k1):
                if KT_sb[m] is None:
                    KT_sb[m] = big.tile([P, S], f32r, name=f"KT{m}", tag=f"KT{m}")
                ps = psA.tile([P, k1 - k0], f32, name="ps_k", tag="psA")
                for c in range(2):
                    nc.tensor.matmul(
                        ps[:, :],
                        w_sb["wkt"][:, c, m * P : (m + 1) * P],
                        xT_sb[c][:, k0:k1],
                        start=(c == 0),
                        stop=(c == 1),
                    )
                nc.vector.tensor_copy(KT_sb[m][:, k0:k1], ps)

            def v_proj(mt):
                # V_aug [key-tile, 4, 65]: per-head 64 value cols + a ones
                # col (whose cd-matmul row is the softmax denominator).
                vt = big.tile([P, 4, 65], f32r, name=f"V{mt}", tag=f"V{mt}")
                ps = psA.tile([P, 512], f32, name="ps_v", tag="psA")
                for c in range(2):
                    nc.tensor.matmul(
                        ps[:, :D],
                        xT_sb[c][:, mt * P : (mt + 1) * P],
                        w_sb["wvt"][:, c, :],
                        start=(c == 0),
                        stop=(c == 1),
                    )
                nc.vector.tensor_copy(
                    vt[:, :, 0:64], ps[:, :D].rearrange("p (h e) -> p h e", h=4)
                )
                nc.vector.tensor_copy(vt[:, :, 64], ones4)
                V_sb[mt] = vt

            # ---- the 64-step attention stream ----
            ets = {}
            ps_cds = {}

            def scores_act(p, f, kt):
                ps_s = psA.tile([P, 1024], f32, name="ps_s", tag="psA")
                for h2 in range(2):
                    nc.tensor.matmul(
                        ps_s[:, h2 * 512 : h2 * 512 + QH],
                        KT_sb[p][64 * h2 : 64 * h2 + 64, kt * P : (kt + 1) * P],
                        QT_sb[p][64 * h2 : 64 * h2 + 64, f * QH : (f + 1) * QH],
                        start=True,
                        stop=True,
                        tile_position=(64 * h2, 0),
                    )
                et = etp.tile([P, 1024], f32r, name="et", tag="et")
                nc.scalar.activation(
                    et, ps_s, Exp, bias=bias_sb[:, kt : kt + 1], scale=SCALE
                )
                ets[(p, f, kt)] = et

            def do_cd(p, f, kt):
                if kt == 0:
                    ps_cds[(p, f)] = psCD.tile(
                        [65, 1024], f32, name="ps_cd", tag="psCD"
                    )
                ps_cd = ps_cds[(p, f)]
                et = ets.pop((p, f, kt))
                for h2 in range(2):
                    h = 2 * p + h2
                    nc.tensor.matmul(
                        ps_cd[0:65, h2 * 512 : h2 * 512 + QH],
                        V_sb[kt][:, h, :],
                        et[:, h2 * 512 : h2 * 512 + QH],
                        start=(kt == 0),
                        stop=(kt == NKT - 1),
                    )
                if kt == NKT - 1:
                    # evict ctx+den right away, freeing the cd PSUM slot for
                    # the next section (whose first cd is DEPTH steps out).
                    cdsb = big.tile(
                        [65, 1024], f32r, name=f"cd{p}{f}", tag=f"cd{p}{f}"
                    )
                    nc.vector.tensor_copy(cdsb, ps_cd)
                    cd_sb[(p, f)] = cdsb
                    if (p, f) == (1, 1):
                        # last section: ScalarE (idle after the last exp)
                        # evicts the den row in parallel with the DVE ctx
                        # copy, so the tail den transpose starts ~1.2us
                        # earlier.
                        dr = big.tile([1, 1024], f32r, name="denrow", tag="denrow")
                        nc.scalar.activation(dr, ps_cd[64:65, :], Copy)
                        cd_sb["denrow"] = dr

            def den_recip(p, f):
                # Transpose the [1,1024] den row into partitions via 8
                # single-row matmuls (2 duplicate output cols each: fp32r
                # ISA needs even free counts), then one [128,16] reciprocal.
                if (p, f) == (1, 1):
                    row, rp = cd_sb["denrow"], 0
                else:
                    row, rp = cd_sb[(p, f)], 64
                ps_den = psA.tile([P, 16], f32, name="ps_den", tag="psA")
                for t in range(8):
                    h2, qq = t // 4, t % 4
                    nc.tensor.matmul(
                        ps_den[:, 2 * t : 2 * t + 2],
                        row[rp : rp + 1, h2 * 512 + qq * P : h2 * 512 + (qq + 1) * P],
                        one1[rp : rp + 1, 0:2],
                        start=True,
                        stop=True,
                    )
                nc.vector.reciprocal(r_sb[(p, f)][:, 0:16], ps_den[:, 0:16])

            def r_ap(m, h):
                f, qq = m // 4, m % 4
                c = 2 * ((h % 2) * 4 + qq)
                return r_sb[(h // 2, f)][:, c : c + 1]

            def oproj_mm2(m, hpair):
                # per-head matmuls for heads (2*hpair, 2*hpair+1), tile m
                f, qq = m // 4, m % 4
                ps2 = psA.tile([P, 2, D], f32, name="ps2", tag="psA")
                for h2 in range(2):
                    h = 2 * hpair + h2
                    nc.tensor.matmul(
                        ps2[:, h2, :],
                        cd_sb[(hpair, f)][
                            0:64, h2 * 512 + qq * P : h2 * 512 + (qq + 1) * P
                        ],
                        wot_sb[:, h, :],
                        start=True,
                        stop=True,
                    )
                return ps2

            def oproj(m):
                # full 4-head output tile (query halves whose cd sections
                # are all evicted): per-head matmuls, then 1/den folded in
                # via per-partition-scalar multiply-adds on DVE.
                ps4 = psA.tile([P, 4, D], f32, name="ps4", tag="psA")
                for h in range(H):
                    p, h2 = h // 2, h % 2
                    nc.tensor.matmul(
                        ps4[:, h, :],
                        cd_sb[(p, m // 4)][
                            0:64, h2 * 512 + (m % 4) * P : h2 * 512 + (m % 4 + 1) * P
                        ],
                        wot_sb[:, h, :],
                        start=True,
                        stop=True,
                    )
                acc = small.tile([P, D], f32, name="acc", tag="acc")
                nc.vector.tensor_scalar_mul(acc, ps4[:, 0, :], r_ap(m, 0))
                for h in range(1, H):
                    dst = (
                        small.tile([P, D], f32, name="acc", tag="acc")
                        if h < H - 1
                        else small.tile([P, D], f32, name="ot", tag="ot", bufs=3)
                    )
                    nc.vector.scalar_tensor_tensor(
                        dst, ps4[:, h, :], r_ap(m, h), acc, Alu.mult, Alu.add
                    )
                    acc = dst
                nc.sync.dma_start(out=out_d[m * P : (m + 1) * P, :], in_=acc)

            accA = {}

            def oproj_a(m):
                # tail tiles, heads 0-1: runs while the last section's cd
                # PSUM is evicted.  ScalarE (idle after the last exp) does
                # the 1/den scaling, GpSimd the add -- DVE stays free.
                ps2 = oproj_mm2(m, 0)
                t0 = small.tile([P, D], f32, name="tA", tag="tA")
                t1 = small.tile([P, D], f32, name="tA", tag="tA")
                nc.scalar.activation(t0, ps2[:, 0, :], Copy, scale=r_ap(m, 0))
                nc.scalar.activation(t1, ps2[:, 1, :], Copy, scale=r_ap(m, 1))
                acc = small.tile([P, D], f32, name=f"accA{m}", tag=f"accA{m}", bufs=1)
                nc.gpsimd.tensor_add(acc, t0, t1)
                accA[m] = acc

            def oproj_b(m):
                # tail tiles, heads 2-3 + final combine + output DMA.
                # Tiles alternate between a DVE fused-multiply-add chain and
                # a ScalarE-scale + GpSimd-add chain so the four final
                # combines run on two engine pipelines concurrently.
                ps2 = oproj_mm2(m, 1)
                ot = small.tile([P, D], f32, name="ot", tag="ot", bufs=3)
                if m % 2 == 0:
                    t = small.tile([P, D], f32, name="acc", tag="acc")
                    nc.vector.scalar_tensor_tensor(
                        t, ps2[:, 0, :], r_ap(m, 2), accA[m], Alu.mult, Alu.add
                    )
                    nc.vector.scalar_tensor_tensor(
                        ot, ps2[:, 1, :], r_ap(m, 3), t, Alu.mult, Alu.add
                    )
                else:
                    t2 = small.tile([P, D], f32, name="tA", tag="tA")
                    t3 = small.tile([P, D], f32, name="tA", tag="tA")
                    nc.scalar.activation(t2, ps2[:, 0, :], Copy, scale=r_ap(m, 2))
                    nc.scalar.activation(t3, ps2[:, 1, :], Copy, scale=r_ap(m, 3))
                    u = small.tile([P, D], f32, name="acc", tag="acc")
                    nc.gpsimd.tensor_add(u, t2, t3)
                    nc.gpsimd.tensor_add(ot, u, accA[m])
                nc.sync.dma_start(out=out_d[m * P : (m + 1) * P, :], in_=ot)

            # ---- prologue: just enough for the first scores steps; dummy
            # warm matmuls pad the PE queue so DMA-wait holes don't re-arm
            # the HAM clock gate ----
            qt_proj(0, 0, 256)
            warm(3)
            qt_proj(0, 256, 512)
            warm(3)
            kt_proj(0, 0, 256)
            warm(3)
            kt_proj(0, 256, 512)
            warm(3)
            v_proj(0)
            warm(2)
            v_proj(1)
            warm(2)

            # Injections keyed by flat scores-step index i (fired after
            # scores_act(i) and do_cd(i-DEPTH)).  Emission-order rules:
            # KT cols for score tile j of section s must be emitted at
            # i <= 16*s + j - 1 (scores_act(j) is emitted at step 16*s+j);
            # V tile for key tile kt of section s at i <= 16*s + kt +
            # DEPTH - 1; den_recip(sec) only after that section's eviction
            # (emitted at step 16*sec + 15 + DEPTH); consumers of r_sb /
            # cd_sb only after the producing den_recip / eviction.
            inj = {
                0: lambda: (v_proj(2), warm(2)),
                1: lambda: (v_proj(3), kt_proj(0, 512, 1024), warm(2)),
                2: lambda: (v_proj(4), warm(2)),
                3: lambda: (v_proj(5), warm(2)),
                4: lambda: (v_proj(6), warm(1)),
                5: lambda: (v_proj(7), kt_proj(0, 1024, 1536), warm(1)),
                6: lambda: v_proj(8),
                7: lambda: v_proj(9),
                8: lambda: v_proj(10),
                9: lambda: (v_proj(11), kt_proj(0, 1536, 2048)),
                10: lambda: v_proj(12),
                11: lambda: (v_proj(13), qt_proj(1, 0, 512)),
                12: lambda: (v_proj(14), kt_proj(1, 0, 512)),
                13: lambda: v_proj(15),
                17: lambda: kt_proj(1, 512, 1024),
                19: lambda: qt_proj(0, 512, 1024),
                21: lambda: kt_proj(1, 1024, 1536),
                23: lambda: den_recip(0, 0),
                25: lambda: kt_proj(1, 1536, 2048),
                35: lambda: qt_proj(1, 512, 1024),
                38: lambda: den_recip(1, 0),
                51: lambda: oproj(0),
                54: lambda: oproj(1),
                55: lambda: den_recip(0, 1),
                57: lambda: oproj(2),
                59: lambda: oproj(3),
                61: lambda: (oproj_a(4), tail_warm(2)),
                62: lambda: tail_warm(2),
                63: lambda: (oproj_a(5), tail_warm(2)),
                64: lambda: (oproj_a(6), tail_warm(2)),
                65: lambda: (oproj_a(7), tail_warm(2)),
                66: lambda: tail_warm(3),
            }

            # cd first within each iteration: a cd whose exp is done must
            # never queue behind a scores matmul that is WAR-blocked on an
            # older ACT freeing its PSUM slot.
            for i in range(64 + DEPTH):
                j = i - DEPTH
                if j >= 0:
                    p, f = SECS[j // 16]
                    do_cd(p, f, j % 16)
                if i < 64:
                    p, f = SECS[i // 16]
                    scores_act(p, f, i % 16)
                if i in inj:
                    inj[i]()

            den_recip(1, 1)
            for m in range(4, 8):
                oproj_b(m)

    nc.compile()
    return nc


def _get_nc():
    if "nc" not in _cache:
        _cache["nc"] = _build_nc()
    return _cache["nc"]


def make_in_maps(x, W_Q, W_K, W_V, W_O, mask):
    wqt = np.ascontiguousarray(W_Q.T).astype(np.float32)
    wkt = np.ascontiguousarray(W_K.T).astype(np.float32)
    wvt = np.ascontiguousarray(W_V.T).astype(np.float32)
    wot = np.ascontiguousarray(W_O.T).astype(np.float32)
    in_maps = []
    for c in range(NCORES):
        b, qh = c // 2, c % 2
        xT_b = np.asarray(x[b]).T.astype(np.float32)
        xT_roll = np.ascontiguousarray(np.roll(xT_b, -qh * QS, axis=1))
        bias = np.where(np.asarray(mask[b]) == 0, -1e30, 0.0).astype(np.float32)
        bias = np.roll(bias, -qh * QS)
        bias = np.ascontiguousarray(bias.reshape(NKT, P).T)
        in_maps.append(
            {
                "xT": xT_roll,
                "wqt": wqt,
                "wkt": wkt,
                "wvt": wvt,
                "wot": wot,
                "bias": bias,
            }
        )
    return in_maps


def gather(results):
    out = np.empty((B, S, D), np.float32)
    for c in range(NCORES):
        b, qh = c // 2, c % 2
        out[b, qh * QS : (qh + 1) * QS, :] = results[c]["out"]
    return out


def kernel(x, W_Q, W_K, W_V, W_O, mask):
    from concourse.bass_utils import run_bass_kernel_spmd

    nc = _get_nc()
    in_maps = make_in_maps(x, W_Q, W_K, W_V, W_O, mask)
    res = run_bass_kernel_spmd(nc, in_maps, core_ids=list(range(NCORES)))
    return gather(res.results)
